# revision 1
# baseline (speedup 1.0000x reference)
"""Trainium2 Bass kernel for nn_Interactor (attention-augmented LSTM).

Problem: B=64, T=512, L=48, DV=DS=H=512.
  per step t: proj_V = x_t W_V^T; proj_R = h W_R^T
              e = tanh(proj_R[:,None,:] + proj_S + proj_V[:,None,:])
              alpha = softmax(e @ w, axis=L); h_ts = alpha @ h_s
              gates = [x_t, h_ts] W_ih^T + h W_hh^T + b; LSTM cell update.

Sharding: data-parallel over batch: 8 cores x 8 batch rows, weights replicated.

Per-core layout ("transposed everything"): feature dims live on SBUF
partitions, the 8 local batch rows on the free dim. State h^T is [512(4x128
chunks), 8]. Recurrent matmuls keep weights stationary (lhsT) and stream the
[128, 8] state as the moving operand. x_t-dependent projections are
precomputed with large matmuls (N=512) into DRAM and streamed back per
T-chunk. Attention softmax runs on a [1, 384] row; the alpha broadcast
across partitions is a single K=1 ones-matmul; the L-reduction is a DVE
blocked tensor_reduce.
"""

import numpy as np

import concourse.bass as bass
import concourse.mybir as mybir
import concourse.tile as tile
from concourse import bacc
from concourse.bass_utils import run_bass_kernel_spmd

F32 = mybir.dt.float32
AF = mybir.ActivationFunctionType
ALU = mybir.AluOpType
AX = mybir.AxisListType

B, T_FULL, L = 64, 512, 48
DV, DS, H = 512, 512, 512
G4 = 4 * H
NCORES = 8
BLOC = B // NCORES  # 8
BL = BLOC * L       # 384
KH = H // 128       # 4 H-chunks
KM = G4 // 128      # 16 gate-row chunks


def build_nc(T=T_FULL, Tc=16, bcast0=True, debug=False):
    """Build the per-core Bass program (SPMD; same program all cores)."""
    assert T % Tc == 0
    nc = bacc.Bacc()

    # ---- DRAM I/O (per-core slices fed via in_maps) ----
    hvT = nc.declare_dram_parameter("hvT", [DV, T * BLOC], F32, isOutput=False)
    hsT = nc.declare_dram_parameter("hsT", [DS, BL], F32, isOutput=False)
    WS_T = nc.declare_dram_parameter("WS_T", [DS, H], F32, isOutput=False)
    WV_T = nc.declare_dram_parameter("WV_T", [DV, H], F32, isOutput=False)
    WihV_T = nc.declare_dram_parameter("WihV_T", [DV, G4], F32, isOutput=False)
    WihS_T = nc.declare_dram_parameter("WihS_T", [DS, G4], F32, isOutput=False)
    Whh_T = nc.declare_dram_parameter("Whh_T", [H, G4], F32, isOutput=False)
    WR_T = nc.declare_dram_parameter("WR_T", [H, H], F32, isOutput=False)
    wvec = nc.declare_dram_parameter("wvec", [H, 1], F32, isOutput=False)
    biasRSV = nc.declare_dram_parameter("biasRSV", [128, KH], F32, isOutput=False)
    biasIH = nc.declare_dram_parameter("biasIH", [128, KM], F32, isOutput=False)
    bw = nc.declare_dram_parameter("bw", [1, 1], F32, isOutput=False)
    out_c = nc.declare_dram_parameter("out_c", [T, KH, 128, BLOC], F32, isOutput=True)

    if debug:
        dbg_rvt = nc.dram_tensor("dbg_rvt", [128, KH * BLOC], F32, kind="ExternalOutput")
        dbg_e = nc.dram_tensor("dbg_e", [128, KH, BL], F32, kind="ExternalOutput")
        dbg_expb = nc.dram_tensor("dbg_expb", [1, BL], F32, kind="ExternalOutput")
        dbg_alpha = nc.dram_tensor("dbg_alpha", [1, BL], F32, kind="ExternalOutput")
        dbg_hts = nc.dram_tensor("dbg_hts", [128, KH * BLOC], F32, kind="ExternalOutput")
        dbg_gates = nc.dram_tensor("dbg_gates", [128, KM * BLOC], F32, kind="ExternalOutput")

    # ---- internal DRAM for precomputed projections ----
    GV_d = nc.dram_tensor("GV_d", [T, KM, 128, BLOC], F32)
    PV_d = nc.dram_tensor("PV_d", [T, KH, 128, BLOC], F32)

    NT = T * BLOC  # hvT free size
    NCW = min(512, NT)  # precompute N-chunk width
    n_nc = NT // NCW

    with tile.TileContext(nc) as tc:
        with (
            tc.tile_pool(name="res", bufs=1) as res,       # resident weights etc
            tc.tile_pool(name="state", bufs=2) as state,   # c state ping-pong
            tc.tile_pool(name="hout", bufs=2) as houtp,    # h ring (8 steps)
            tc.tile_pool(name="stream", bufs=2) as stream, # GV/PV chunks
            tc.tile_pool(name="work", bufs=2) as work,     # per-step tiles
        ):
            # ---------- resident loads ----------
            wr_sb = res.tile([128, KH, H], F32, tag="wr")
            nc.sync.dma_start(out=wr_sb, in_=WR_T.rearrange("(k p) m -> p k m", p=128))
            wihs_sb = res.tile([128, KH, G4], F32, tag="wihs")
            nc.sync.dma_start(out=wihs_sb, in_=WihS_T.rearrange("(k p) m -> p k m", p=128))
            whh_sb = res.tile([128, KH, G4], F32, tag="whh")
            nc.sync.dma_start(out=whh_sb, in_=Whh_T.rearrange("(k p) m -> p k m", p=128))
            hs_sb = res.tile([128, KH, BL], F32, tag="hs")
            nc.sync.dma_start(out=hs_sb, in_=hsT.rearrange("(k p) n -> p k n", p=128))
            wvec_sb = res.tile([128, KH], F32, tag="wvec")
            nc.sync.dma_start(out=wvec_sb, in_=wvec.rearrange("(k p) o -> p (k o)", p=128))
            brsv_sb = res.tile([128, KH], F32, tag="brsv")
            nc.sync.dma_start(out=brsv_sb, in_=biasRSV[:, :])
            bih_sb = res.tile([128, KM], F32, tag="bih")
            nc.sync.dma_start(out=bih_sb, in_=biasIH[:, :])
            bw_sb = res.tile([1, 1], F32, tag="bw")
            nc.sync.dma_start(out=bw_sb, in_=bw[:, :])
            ones_sb = res.tile([1, 128], F32, tag="ones")
            nc.vector.memset(ones_sb, 1.0)
            # PS: proj_S + (b_S+b_R+b_V), [128, KH, BL]
            ps_sb = res.tile([128, KH, BL], F32, tag="ps")

            # ---------- precompute phase ----------
            with (
                tc.tile_pool(name="prew", bufs=1) as prew,
                tc.tile_pool(name="prehv", bufs=4) as prehv,
                tc.tile_pool(name="prestg", bufs=2) as prestg,
                tc.tile_pool(name="prepsum", bufs=4, space="PSUM") as prepsum,
            ):
                ws_sb = prew.tile([128, KH, H], F32, tag="ws")
                nc.sync.dma_start(out=ws_sb, in_=WS_T.rearrange("(k p) m -> p k m", p=128))
                wv_sb = prew.tile([128, KH, H], F32, tag="wv")
                nc.sync.dma_start(out=wv_sb, in_=WV_T.rearrange("(k p) m -> p k m", p=128))
                wihv_sb = prew.tile([128, KH, G4], F32, tag="wihv")
                nc.sync.dma_start(out=wihv_sb, in_=WihV_T.rearrange("(k p) m -> p k m", p=128))

                # PS = W_S @ hsT + biasRSV
                for m in range(KH):
                    pps = prepsum.tile([128, 512], F32, tag="pp")
                    for kc in range(KH):
                        nc.tensor.matmul(
                            pps[:, :BL],
                            ws_sb[:, kc, m * 128:(m + 1) * 128],
                            hs_sb[:, kc, :],
                            start=(kc == 0), stop=(kc == KH - 1),
                        )
                    nc.vector.tensor_scalar_add(ps_sb[:, m, :], pps[:, :BL], brsv_sb[:, m:m + 1])

                # PV / GV over hvT N-chunks of 512
                for ncnk in range(n_nc):
                    nsl = slice(ncnk * NCW, (ncnk + 1) * NCW)
                    hv_t = []
                    for kc in range(KH):
                        t_ = prehv.tile([128, NCW], F32, tag="hv")
                        nc.sync.dma_start(
                            out=t_, in_=hvT[kc * 128:(kc + 1) * 128, nsl])
                        hv_t.append(t_)
                    # PV chunks: out rows m*128, cols (t,b)
                    for m in range(KH):
                        ppv = prepsum.tile([128, NCW], F32, tag="pp")
                        for kc in range(KH):
                            nc.tensor.matmul(
                                ppv, wv_sb[:, kc, m * 128:(m + 1) * 128],
                                hv_t[kc], start=(kc == 0), stop=(kc == KH - 1))
                        stg = prestg.tile([128, NCW], F32, tag="pvstg")
                        nc.scalar.copy(stg, ppv)
                        # dest: PV_d[t0:t0+tw, m, :, :]  dims [t,p,b] <- src [p,(t,b)]
                        t0 = ncnk * NCW // BLOC
                        tw = NCW // BLOC
                        nc.sync.dma_start(
                            out=PV_d[t0:t0 + tw, m, :, :].rearrange("t p b -> p t b"),
                            in_=stg.rearrange("p (t b) -> p t b", b=BLOC))
                    # GV chunks (+ biasIH fold)
                    for m in range(KM):
                        pgv = prepsum.tile([128, NCW], F32, tag="pp")
                        for kc in range(KH):
                            nc.tensor.matmul(
                                pgv, wihv_sb[:, kc, m * 128:(m + 1) * 128],
                                hv_t[kc], start=(kc == 0), stop=(kc == KH - 1))
                        stg = prestg.tile([128, NCW], F32, tag="gvstg")
                        nc.vector.tensor_scalar_add(stg, pgv, bih_sb[:, m:m + 1])
                        t0 = ncnk * NCW // BLOC
                        tw = NCW // BLOC
                        nc.sync.dma_start(
                            out=GV_d[t0:t0 + tw, m, :, :].rearrange("t p b -> p t b"),
                            in_=stg.rearrange("p (t b) -> p t b", b=BLOC))

            # ---------- recurrence ----------
            psum = tc.alloc_tile_pool(name="psum", bufs=2, space="PSUM")
            czero = state.tile([128, 32], F32, tag="c")
            nc.vector.memset(czero, 0.0)
            hzero = res.tile([128, 32], F32, tag="h0")
            nc.vector.memset(hzero, 0.0)
            c_prev = czero
            h_prev = hzero  # [128, (kc,b)]

            HB = 8  # h ring steps per DMA
            gv_cur = pv_cur = None
            hbuf = None

            for t in range(T):
                ic = t % Tc
                if ic == 0:
                    gv_cur = stream.tile([128, Tc, KM, BLOC], F32, tag="gv")
                    nc.sync.dma_start(
                        out=gv_cur,
                        in_=GV_d[t:t + Tc].rearrange("t m p b -> p t m b"))
                    pv_cur = stream.tile([128, Tc, KH, BLOC], F32, tag="pv")
                    nc.sync.dma_start(
                        out=pv_cur,
                        in_=PV_d[t:t + Tc].rearrange("t k p b -> p t k b"))
                ts_ = t % HB
                if ts_ == 0:
                    hbuf = houtp.tile([128, HB, KH, BLOC], F32, tag="hb")

                # 1. proj_R -> psum_rv [128, (kc,b)]
                psum_rv = psum.tile([128, KH, BLOC], F32, tag="rv")
                for m in range(KH):
                    for kc in range(KH):
                        nc.tensor.matmul(
                            psum_rv[:, m, :],
                            wr_sb[:, kc, m * 128:(m + 1) * 128],
                            h_prev[:, kc * BLOC:(kc + 1) * BLOC],
                            start=(kc == 0), stop=(kc == KH - 1))
                # 2. rvt = psum_rv + PV[t]
                rvt = work.tile([128, KH * BLOC], F32, tag="rvt")
                nc.vector.tensor_tensor(
                    rvt.rearrange("p (k b) -> p k b", b=BLOC),
                    psum_rv, pv_cur[:, ic], ALU.add)
                # 3. e = tanh(PS + bcast_L(rvt))  [128, KH, BL]
                e_all = work.tile([128, KH, BL], F32, tag="e")
                for kc in range(KH):
                    if bcast0:
                        sl = rvt[:, kc * BLOC:(kc + 1) * BLOC]
                        bc = bass.AP(tensor=sl.tensor, offset=sl.offset,
                                     ap=[sl.ap[0], [1, BLOC], [0, L]])
                        nc.vector.tensor_tensor(
                            e_all[:, kc].rearrange("p (b l) -> p b l", l=L),
                            hs_ps_view(ps_sb, kc),
                            bc, ALU.add)
                    else:
                        for b in range(BLOC):
                            nc.vector.tensor_scalar_add(
                                e_all[:, kc, b * L:(b + 1) * L],
                                ps_sb[:, kc, b * L:(b + 1) * L],
                                rvt[:, kc * BLOC + b:kc * BLOC + b + 1])
                    nc.scalar.activation(e_all[:, kc], e_all[:, kc], AF.Tanh)
                # 4. beta = wvec . e  -> psum [1, BL]
                psum_beta = psum.tile([1, BL], F32, tag="beta")
                for kc in range(KH):
                    nc.tensor.matmul(
                        psum_beta, wvec_sb[:, kc:kc + 1], e_all[:, kc],
                        start=(kc == 0), stop=(kc == KH - 1))
                # 5. softmax over l (48) per b; no max-subtraction needed
                expb = work.tile([1, BL], F32, tag="expb")
                nc.scalar.activation(expb, psum_beta, AF.Exp, bias=bw_sb[:, 0:1])
                denom = work.tile([1, BLOC], F32, tag="denom")
                nc.vector.tensor_reduce(
                    denom, expb.rearrange("p (b l) -> p b l", l=L), AX.X, ALU.add)
                rec = work.tile([1, BLOC], F32, tag="rec")
                nc.vector.reciprocal(rec, denom)
                alpha = work.tile([1, BL], F32, tag="alpha")
                if bcast0:
                    rbc = bass.AP(tensor=rec.tensor, offset=rec.offset,
                                  ap=[rec.ap[0], [1, BLOC], [0, L]])
                    nc.vector.tensor_tensor(
                        alpha.rearrange("p (b l) -> p b l", l=L),
                        expb.rearrange("p (b l) -> p b l", l=L), rbc, ALU.mult)
                else:
                    for b in range(BLOC):
                        nc.vector.tensor_scalar_mul(
                            alpha[:, b * L:(b + 1) * L],
                            expb[:, b * L:(b + 1) * L], rec[:, b:b + 1])
                # 6. alpha_bc via ones-matmul; apply -> h_tsT [128, (kc,b)]
                psum_abc = psum.tile([128, BL], F32, tag="abc")
                nc.tensor.matmul(psum_abc, ones_sb, alpha, start=True, stop=True)
                h_ts = work.tile([128, KH * BLOC], F32, tag="hts")
                for kc in range(KH):
                    prod = work.tile([128, BL], F32, tag="prod")
                    nc.vector.tensor_tensor(prod, psum_abc, hs_sb[:, kc, :], ALU.mult)
                    nc.vector.tensor_reduce(
                        h_ts[:, kc * BLOC:(kc + 1) * BLOC],
                        prod.rearrange("p (b l) -> p b l", l=L), AX.X, ALU.add)
                # 7. gates = WihS @ h_ts + Whh @ h_prev (+GV via DVE)
                psum_g = psum.tile([128, KM, BLOC], F32, tag="g")
                for m in range(KM):
                    for kc in range(KH):
                        nc.tensor.matmul(
                            psum_g[:, m, :],
                            wihs_sb[:, kc, m * 128:(m + 1) * 128],
                            h_ts[:, kc * BLOC:(kc + 1) * BLOC],
                            start=(kc == 0), stop=False)
                    for kc in range(KH):
                        nc.tensor.matmul(
                            psum_g[:, m, :],
                            whh_sb[:, kc, m * 128:(m + 1) * 128],
                            h_prev[:, kc * BLOC:(kc + 1) * BLOC],
                            start=False, stop=(kc == KH - 1))
                gates = work.tile([128, KM * BLOC], F32, tag="gates")
                nc.vector.tensor_tensor(
                    gates.rearrange("p (m b) -> p m b", b=BLOC),
                    psum_g, gv_cur[:, ic], ALU.add)
                # 8. LSTM pointwise; gate m-chunks: 0-3=i, 4-7=f, 8-11=g, 12-15=o
                gi = gates[:, 0:32]
                gf = gates[:, 32:64]
                gg = gates[:, 64:96]
                go = gates[:, 96:128]
                si = work.tile([128, 32], F32, tag="si")
                nc.scalar.activation(si, gi, AF.Sigmoid)
                sf = work.tile([128, 32], F32, tag="sf")
                nc.scalar.activation(sf, gf, AF.Sigmoid)
                tg = work.tile([128, 32], F32, tag="tg")
                nc.scalar.activation(tg, gg, AF.Tanh)
                so = work.tile([128, 32], F32, tag="so")
                nc.scalar.activation(so, go, AF.Sigmoid)
                t1 = work.tile([128, 32], F32, tag="t1")
                nc.vector.tensor_tensor(t1, si, tg, ALU.mult)
                t2 = work.tile([128, 32], F32, tag="t2")
                nc.vector.tensor_tensor(t2, sf, c_prev, ALU.mult)
                c_new = state.tile([128, 32], F32, tag="c")
                nc.vector.tensor_tensor(c_new, t1, t2, ALU.add)
                tc_ = work.tile([128, 32], F32, tag="tc")
                nc.scalar.activation(tc_, c_new, AF.Tanh)
                h_new = hbuf[:, ts_]  # [128, KH, BLOC] view
                nc.vector.tensor_tensor(
                    h_new.rearrange("p k b -> p (k b)"), so, tc_, ALU.mult)
                if debug and t == 0:
                    nc.sync.dma_start(out=dbg_rvt[:, :], in_=rvt)
                    nc.sync.dma_start(out=dbg_e[:, :, :], in_=e_all)
                    nc.sync.dma_start(out=dbg_expb[:, :], in_=expb)
                    nc.sync.dma_start(out=dbg_alpha[:, :], in_=alpha)
                    nc.sync.dma_start(out=dbg_hts[:, :], in_=h_ts)
                    nc.sync.dma_start(out=dbg_gates[:, :], in_=gates)
                c_prev = c_new
                h_prev = h_new.rearrange("p k b -> p (k b)")
                # 9. flush h ring every HB steps
                if ts_ == HB - 1 or t == T - 1:
                    nb = ts_ + 1
                    t0 = t - nb + 1
                    nc.sync.dma_start(
                        out=out_c[t0:t0 + nb].rearrange("t k p b -> p (t k) b"),
                        in_=hbuf[:, :nb].rearrange("p t k b -> p (t k) b"))
            psum.release()
    nc.finalize()
    return nc


def hs_ps_view(ps_sb, kc):
    v = ps_sb[:, kc, :]
    return v.rearrange("p (b l) -> p b l", l=L)


# ---------------- host side ----------------

def prep_core_inputs(h_v, h_s, W, T=T_FULL):
    """Per-core input maps. W: dict of full weight arrays."""
    WS_T = np.ascontiguousarray(W["W_S"].T)
    WV_T = np.ascontiguousarray(W["W_V"].T)
    WihV_T = np.ascontiguousarray(W["W_ih"][:, :DV].T)
    WihS_T = np.ascontiguousarray(W["W_ih"][:, DV:].T)
    Whh_T = np.ascontiguousarray(W["W_hh"].T)
    WR_T = np.ascontiguousarray(W["W_R"].T)
    wvec = np.ascontiguousarray(W["W_w"][0][:, None])
    biasRSV = np.ascontiguousarray(
        (W["b_S"] + W["b_R"] + W["b_V"]).reshape(KH, 128).T)
    biasIH = np.ascontiguousarray((W["b_ih"] + W["b_hh"]).reshape(KM, 128).T)
    bw = np.ascontiguousarray(W["b_w"].reshape(1, 1))
    maps = []
    for c in range(NCORES):
        bs = slice(c * BLOC, (c + 1) * BLOC)
        hvT = np.ascontiguousarray(
            h_v[bs, :T].transpose(2, 1, 0).reshape(DV, T * BLOC))
        hsT = np.ascontiguousarray(
            h_s[bs].transpose(2, 0, 1).reshape(DS, BLOC * L))
        maps.append({
            "hvT": hvT, "hsT": hsT, "WS_T": WS_T, "WV_T": WV_T,
            "WihV_T": WihV_T, "WihS_T": WihS_T, "Whh_T": Whh_T, "WR_T": WR_T,
            "wvec": wvec, "biasRSV": biasRSV, "biasIH": biasIH, "bw": bw,
        })
    return maps


_NC_CACHE = {}


def kernel(**inputs):
    h_v = np.asarray(inputs["h_v"], dtype=np.float32)
    h_s = np.asarray(inputs["h_s"], dtype=np.float32)
    W = {k: np.asarray(v, dtype=np.float32) for k, v in inputs.items()}
    key = "full"
    if key not in _NC_CACHE:
        _NC_CACHE[key] = build_nc(T=T_FULL, Tc=16, bcast0=True)
    nc = _NC_CACHE[key]
    maps = prep_core_inputs(h_v, h_s, W, T=T_FULL)
    res = run_bass_kernel_spmd(nc, maps, list(range(NCORES)))
    outs = []
    for c in range(NCORES):
        arr = res.results[c]["out_c"]  # [T, KH, 128, BLOC]
        outs.append(np.ascontiguousarray(
            arr.transpose(3, 0, 1, 2).reshape(BLOC, T_FULL, H)))
    return np.concatenate(outs, axis=0).astype(np.float32)


if __name__ == "__main__":
    # smoke build
    nc = build_nc(T=8, Tc=4)
    print("built ok:", len(nc.m.functions[0].instructions) if hasattr(nc.m.functions[0], 'instructions') else "?")



# revision 2
# speedup vs baseline: 1.0537x; 1.0537x over previous
"""Trainium2 Bass kernel v2 for nn_Interactor (attention-augmented LSTM).

Key changes vs v1:
- fp16 weights/streams/elementwise (PE: 1c/row + FWL weight loads = 2x;
  DVE: 2x packed modes). PSUM + LSTM cell state stay fp32.
  (numpy sim: rel err 4.4e-4 vs 2e-2 gate)
- sigmoid via tanh (sigmoid(x) = 0.5 + 0.5*tanh(x/2)) so every Act op uses
  the exp_and_others table: zero 1.3us act-table reloads per step.
- h_ts contraction on the PE: h_s resident in block-diag [(b,l64), DS]
  layout; per-step exp(beta) row is transposed onto partitions with 8 tiny
  PE transposes into a memset-once PSUM region, then 16 [128x128] matmuls
  with N=2 produce h_ts^T directly. Softmax normalization folded in at the
  end (1/denom broadcast via K=1 ones-matmul).
- PE emission order per step: W_R(16) | Whh part A(32) | beta(4) |
  Whh part B(32) | transposes(8) | rec(1) | hts(16) | WihS(64) so the PE
  never stalls long on the attention chain.
- gate order permuted on host to (i, f, o, g) so one Act call covers the
  three sigmoids.

Layout: feature dims on partitions, 8 local batch rows on free dims,
h state [128, (kc, b)] fp16.
"""

import numpy as np

import concourse.bass as bass
import concourse.mybir as mybir
import concourse.tile as tile
from concourse import bacc
from concourse.bass_utils import run_bass_kernel_spmd

F32 = mybir.dt.float32
F16 = mybir.dt.float16
AF = mybir.ActivationFunctionType
ALU = mybir.AluOpType
AX = mybir.AxisListType

B, T_FULL, L = 64, 512, 48
DV, DS, H = 512, 512, 512
G4 = 4 * H
NCORES = 8
BLOC = B // NCORES  # 8
BL = BLOC * L       # 384
KH = H // 128       # 4
KM = G4 // 128      # 16
L64 = 64            # padded L for block-diag hs layout


def build_nc(T=T_FULL, Tc=16, debug=False, dbg_t=0):
    assert T % Tc == 0
    nc = bacc.Bacc()

    # ---- DRAM I/O ----
    hvT = nc.declare_dram_parameter("hvT", [DV, T * BLOC], F16, isOutput=False)
    hsT = nc.declare_dram_parameter("hsT", [DS, BL], F16, isOutput=False)
    hs_bT = nc.declare_dram_parameter("hs_bT", [L, BLOC, DS], F16, isOutput=False)
    WS_T = nc.declare_dram_parameter("WS_T", [DS, H], F16, isOutput=False)
    WV_T = nc.declare_dram_parameter("WV_T", [DV, H], F16, isOutput=False)
    WihV_T = nc.declare_dram_parameter("WihV_T", [DV, G4], F16, isOutput=False)
    WihS_T = nc.declare_dram_parameter("WihS_T", [DS, G4], F16, isOutput=False)
    Whh_T = nc.declare_dram_parameter("Whh_T", [H, G4], F16, isOutput=False)
    WR_T = nc.declare_dram_parameter("WR_T", [H, H], F16, isOutput=False)
    wvec = nc.declare_dram_parameter("wvec", [H, 1], F16, isOutput=False)
    biasRSV = nc.declare_dram_parameter("biasRSV", [128, KH], F32, isOutput=False)
    biasIH = nc.declare_dram_parameter("biasIH", [128, KM], F32, isOutput=False)
    bw = nc.declare_dram_parameter("bw", [1, 1], F32, isOutput=False)
    out_c = nc.declare_dram_parameter("out_c", [128, T, KH, BLOC], F32, isOutput=True)

    if debug:
        dbg_rvt = nc.dram_tensor("dbg_rvt", [128, KH * BLOC], F16, kind="ExternalOutput")
        dbg_e = nc.dram_tensor("dbg_e", [128, KH, BL], F16, kind="ExternalOutput")
        dbg_expb = nc.dram_tensor("dbg_expb", [1, BL], F32, kind="ExternalOutput")
        dbg_rhsbd = nc.dram_tensor("dbg_rhsbd", [L, BLOC], F16, kind="ExternalOutput")
        dbg_hts = nc.dram_tensor("dbg_hts", [128, KH * BLOC], F16, kind="ExternalOutput")
        dbg_gates = nc.dram_tensor("dbg_gates", [128, KM * BLOC], F32, kind="ExternalOutput")
        dbg_hprev = nc.dram_tensor("dbg_hprev", [128, KH * BLOC], F16, kind="ExternalOutput")
        dbg_cprev = nc.dram_tensor("dbg_cprev", [128, KH * BLOC], F32, kind="ExternalOutput")

    # internal DRAM for precomputed x_t projections (fp16)
    GV_d = nc.dram_tensor("GV_d", [128, T, KM, BLOC], F16)
    PV_d = nc.dram_tensor("PV_d", [128, T, KH, BLOC], F16)

    NT = T * BLOC
    NCW = min(512, NT)
    n_nc = NT // NCW

    with tile.TileContext(nc) as tc:
        with (
            tc.tile_pool(name="res", bufs=1) as res,
            tc.tile_pool(name="state", bufs=2) as state,
            tc.tile_pool(name="hout", bufs=2) as houtp,
            tc.tile_pool(name="stream", bufs=2) as stream,
            tc.tile_pool(name="work", bufs=2) as work,
            tc.tile_pool(name="ppersist", bufs=1, space="PSUM") as ppersist,
        ):
            # ---------- resident loads ----------
            wr_sb = res.tile([128, KH, H], F16, tag="wr")
            nc.sync.dma_start(out=wr_sb, in_=WR_T.rearrange("(k p) m -> p k m", p=128))
            wihs_sb = res.tile([128, KH, G4], F16, tag="wihs")
            nc.sync.dma_start(out=wihs_sb, in_=WihS_T.rearrange("(k p) m -> p k m", p=128))
            whh_sb = res.tile([128, KH, G4], F16, tag="whh")
            nc.sync.dma_start(out=whh_sb, in_=Whh_T.rearrange("(k p) m -> p k m", p=128))
            hs_sb = res.tile([128, KH, BL], F16, tag="hs")
            nc.sync.dma_start(out=hs_sb, in_=hsT.rearrange("(k p) n -> p k n", p=128))
            hsb_sb = res.tile([L, BLOC, DS], F16, tag="hsb")
            nc.sync.dma_start(out=hsb_sb, in_=hs_bT[:, :, :])
            wvec_sb = res.tile([128, KH], F16, tag="wvec")
            nc.sync.dma_start(out=wvec_sb, in_=wvec.rearrange("(k p) o -> p (k o)", p=128))
            brsv_sb = res.tile([128, KH], F32, tag="brsv")
            nc.sync.dma_start(out=brsv_sb, in_=biasRSV[:, :])
            bih_sb = res.tile([128, KM], F32, tag="bih")
            nc.sync.dma_start(out=bih_sb, in_=biasIH[:, :])
            bw_sb = res.tile([1, 1], F32, tag="bw")
            nc.sync.dma_start(out=bw_sb, in_=bw[:, :])
            ones1 = res.tile([1, 128], F16, tag="ones1")
            nc.vector.memset(ones1, 1.0)
            ones48 = res.tile([L, 1], F16, tag="ones48")
            nc.vector.memset(ones48, 1.0)
            ident11 = res.tile([1, 1], F32, tag="ident11")
            nc.vector.memset(ident11, 1.0)
            ps_sb = res.tile([128, KH, BL], F16, tag="ps")


            # ---------- precompute phase ----------
            with (
                tc.tile_pool(name="prew", bufs=1) as prew,
                tc.tile_pool(name="prehv", bufs=4) as prehv,
                tc.tile_pool(name="prestg", bufs=3) as prestg,
                tc.tile_pool(name="prepsum", bufs=4, space="PSUM") as prepsum,
            ):
                ws_sb = prew.tile([128, KH, H], F16, tag="ws")
                nc.sync.dma_start(out=ws_sb, in_=WS_T.rearrange("(k p) m -> p k m", p=128))
                wv_sb = prew.tile([128, KH, H], F16, tag="wv")
                nc.sync.dma_start(out=wv_sb, in_=WV_T.rearrange("(k p) m -> p k m", p=128))
                wihv_sb = prew.tile([128, KH, G4], F16, tag="wihv")
                nc.sync.dma_start(out=wihv_sb, in_=WihV_T.rearrange("(k p) m -> p k m", p=128))

                # PS = W_S @ hsT + biasRSV  (stored fp16)
                for m in range(KH):
                    pps = prepsum.tile([128, 512], F32, tag="pp")
                    for kc in range(KH):
                        nc.tensor.matmul(
                            pps[:, :BL],
                            ws_sb[:, kc, m * 128:(m + 1) * 128],
                            hs_sb[:, kc, :],
                            start=(kc == 0), stop=(kc == KH - 1),
                        )
                    nc.vector.tensor_scalar_add(ps_sb[:, m, :], pps[:, :BL], brsv_sb[:, m:m + 1])

                for ncnk in range(n_nc):
                    nsl = slice(ncnk * NCW, (ncnk + 1) * NCW)
                    hv_t = []
                    for kc in range(KH):
                        t_ = prehv.tile([128, NCW], F16, tag="hv")
                        nc.sync.dma_start(out=t_, in_=hvT[kc * 128:(kc + 1) * 128, nsl])
                        hv_t.append(t_)
                    t0 = ncnk * NCW // BLOC
                    tw = NCW // BLOC
                    stg_pv = prestg.tile([128, tw, KH, BLOC], F16, tag="pvstg")
                    for m in range(KH):
                        ppv = prepsum.tile([128, NCW], F32, tag="pp")
                        for kc in range(KH):
                            nc.tensor.matmul(
                                ppv, wv_sb[:, kc, m * 128:(m + 1) * 128],
                                hv_t[kc], start=(kc == 0), stop=(kc == KH - 1))
                        nc.vector.tensor_copy(
                            stg_pv[:, :, m, :],
                            ppv.rearrange("p (t b) -> p t b", b=BLOC))
                    nc.sync.dma_start(out=PV_d[:, t0:t0 + tw, :, :], in_=stg_pv)
                    stg_gv = prestg.tile([128, tw, KM, BLOC], F16, tag="gvstg")
                    for m in range(KM):
                        pgv = prepsum.tile([128, NCW], F32, tag="pp")
                        for kc in range(KH):
                            nc.tensor.matmul(
                                pgv, wihv_sb[:, kc, m * 128:(m + 1) * 128],
                                hv_t[kc], start=(kc == 0), stop=(kc == KH - 1))
                        pgv3 = pgv.rearrange("p (t b) -> p t b", b=BLOC)
                        if m % 2 == 0:
                            nc.vector.tensor_scalar_add(
                                stg_gv[:, :, m, :], pgv3, bih_sb[:, m:m + 1])
                        else:
                            nc.scalar.activation(
                                stg_gv[:, :, m, :], pgv3, AF.Identity,
                                bias=bih_sb[:, m:m + 1])
                    nc.sync.dma_start(out=GV_d[:, t0:t0 + tw, :, :], in_=stg_gv)

            # ---------- recurrence ----------
            psum = tc.alloc_tile_pool(name="psum", bufs=2, space="PSUM")
            psumg_pool = tc.alloc_tile_pool(name="psumg", bufs=2, space="PSUM")
            czero = state.tile([128, 32], F32, tag="c")
            nc.vector.memset(czero, 0.0)
            hzero = res.tile([128, 32], F16, tag="h0")
            nc.vector.memset(hzero, 0.0)
            c_prev = czero
            h_prev = hzero

            HB = 8
            gv_cur = pv_cur = None
            hbuf = None

            for t in range(T):
                ic = t % Tc
                if ic == 0:
                    gv_cur = stream.tile([128, Tc, KM, BLOC], F16, tag="gv")
                    nc.sync.dma_start(out=gv_cur, in_=GV_d[:, t:t + Tc, :, :])
                    pv_cur = stream.tile([128, Tc, KH, BLOC], F16, tag="pv")
                    nc.sync.dma_start(out=pv_cur, in_=PV_d[:, t:t + Tc, :, :])
                ts_ = t % HB
                if ts_ == 0:
                    hbuf = houtp.tile([128, HB, KH, BLOC], F32, tag="hb")

                # --- PE: proj_R (needs h_prev) ---
                # one packed psum bank: rv [0:32), rec128 [32:40), hts [40:72),
                # beta on partition 0 cols [128:512)
                psumA = psum.tile([128, 512], F32, tag="pa")
                psum_rv = psumA[:, 0:32].rearrange("p (k b) -> p k b", b=BLOC)
                for m in range(KH):
                    for kc in range(KH):
                        nc.tensor.matmul(
                            psum_rv[:, m, :],
                            wr_sb[:, kc, m * 128:(m + 1) * 128],
                            h_prev[:, kc * BLOC:(kc + 1) * BLOC],
                            start=(kc == 0), stop=(kc == KH - 1))
                # --- PE: Whh part A (m 0..7) ---
                psum_g = psumg_pool.tile([128, KM, BLOC], F32, tag="g")
                psum_g2 = psumg_pool.tile([128, KM, BLOC], F32, tag="g2")
                for m in range(KM // 2):
                    for kc in range(KH):
                        nc.tensor.matmul(
                            psum_g[:, m, :],
                            whh_sb[:, kc, m * 128:(m + 1) * 128],
                            h_prev[:, kc * BLOC:(kc + 1) * BLOC],
                            start=(kc == 0), stop=(kc == KH - 1))

                # --- per-kc pipeline: rvt_kc -> e-add_kc -> tanh_kc (beta below) ---
                rvt = work.tile([128, KH * BLOC], F16, tag="rvt")
                e_all = work.tile([128, KH, BL], F16, tag="e")
                for kc in range(KH):
                    nc.vector.tensor_tensor(
                        rvt[:, kc * BLOC:(kc + 1) * BLOC],
                        psum_rv[:, kc, :], pv_cur[:, ic, kc, :], ALU.add)
                    sl = rvt[:, kc * BLOC:(kc + 1) * BLOC]
                    bck = bass.AP(tensor=sl.tensor, offset=sl.offset,
                                  ap=[sl.ap[0], [1, BLOC], [0, L]])
                    nc.vector.tensor_tensor(
                        e_all[:, kc].rearrange("p (b l) -> p b l", l=L),
                        ps_sb[:, kc].rearrange("p (b l) -> p b l", l=L),
                        bck, ALU.add)
                    nc.scalar.activation(e_all[:, kc], e_all[:, kc], AF.Tanh)

                # --- PE: beta (emitted after WhhA; per-kc inputs arrive pipelined) ---
                psum_beta = psumA[0:1, 128:128 + BL]
                for kc in range(KH):
                    nc.tensor.matmul(
                        psum_beta, wvec_sb[:, kc:kc + 1], e_all[:, kc],
                        start=(kc == 0), stop=(kc == KH - 1))
                # --- PE: Whh part B (m 8..15) ---
                for m in range(KM // 2, KM):
                    for kc in range(KH):
                        nc.tensor.matmul(
                            psum_g[:, m, :],
                            whh_sb[:, kc, m * 128:(m + 1) * 128],
                            h_prev[:, kc * BLOC:(kc + 1) * BLOC],
                            start=(kc == 0), stop=(kc == KH - 1))

                # --- softmax: exp, transpose strips, denom via ones-matmul ---
                expb = work.tile([1, BL], F32, tag="expb")
                nc.scalar.activation(expb, psum_beta, AF.Exp, bias=bw_sb[:, 0:1])
                psum_expT = psumA[0:L, 72:72 + BLOC]
                for c_ in range(BLOC):
                    nc.tensor.transpose(
                        psum_expT[:, c_:c_ + 1],
                        expb[:, L * c_:L * (c_ + 1)],
                        ident11)
                rhs_bd = work.tile([L, BLOC], F16, tag="rhsbd")
                nc.vector.tensor_copy(rhs_bd, psum_expT)
                psum_den = psumA[0:1, 120:128]
                nc.tensor.matmul(psum_den, ones48, rhs_bd, start=True, stop=True)
                # --- PE: h_ts matmuls (hs per-b stationary) ---
                psum_hts = psumA[:, 40:72].rearrange("p (k b) -> p k b", b=BLOC)
                for b_ in range(BLOC):
                    for kc in range(KH):
                        nc.tensor.matmul(
                            psum_hts[:, kc, b_:b_ + 1],
                            hsb_sb[:, b_, kc * 128:(kc + 1) * 128],
                            rhs_bd[:, b_:b_ + 1],
                            start=True, stop=True)
                rec = work.tile([1, BLOC], F16, tag="rec")
                with nc.allow_low_precision(reason="softmax 1/denom in fp16 is ample"):
                    nc.vector.reciprocal(rec, psum_den)
                psum_rec = psumA[:, 32:40]
                nc.tensor.matmul(psum_rec, ones1, rec, start=True, stop=True)
                rec128_sb = work.tile([128, BLOC], F16, tag="rec128sb")
                nc.vector.tensor_copy(rec128_sb, psum_rec)
                h_ts = work.tile([128, KH * BLOC], F16, tag="hts")
                rb = bass.AP(tensor=rec128_sb.tensor, offset=rec128_sb.offset,
                             ap=[rec128_sb.ap[0], [0, KH], [1, BLOC]])
                nc.vector.tensor_tensor(
                    h_ts.rearrange("p (k b) -> p k b", b=BLOC),
                    psum_hts, rb, ALU.mult)

                # --- PE: WihS @ h_ts (finishes gates) ---
                for m in range(KM):
                    for kc in range(KH):
                        nc.tensor.matmul(
                            psum_g2[:, m, :],
                            wihs_sb[:, kc, m * 128:(m + 1) * 128],
                            h_ts[:, kc * BLOC:(kc + 1) * BLOC],
                            start=(kc == 0), stop=(kc == KH - 1))
                g1 = work.tile([128, KM * BLOC], F32, tag="g1")
                nc.vector.tensor_tensor(
                    g1.rearrange("p (m b) -> p m b", b=BLOC),
                    psum_g, gv_cur[:, ic], ALU.add)
                gates = work.tile([128, KM * BLOC], F32, tag="gates")
                nc.vector.tensor_tensor(
                    gates.rearrange("p (m b) -> p m b", b=BLOC),
                    g1.rearrange("p (m b) -> p m b", b=BLOC), psum_g2, ALU.add)

                # --- LSTM pointwise; cols (i,f,o,g); g-rows pre-doubled so one
                # Act call computes tanh(i/2), tanh(f/2), tanh(o/2), tanh(g) ---
                ts_a = work.tile([128, 128], F32, tag="tsa")
                nc.scalar.activation(ts_a, gates, AF.Tanh, scale=0.5)
                sfc = work.tile([128, 32], F32, tag="sfc")
                nc.vector.scalar_tensor_tensor(
                    sfc, ts_a[:, 32:64], 1.0, c_prev, ALU.add, ALU.mult)
                sig = work.tile([128, 32], F32, tag="sig")
                nc.vector.scalar_tensor_tensor(
                    sig, ts_a[:, 0:32], 1.0, ts_a[:, 96:128], ALU.add, ALU.mult)
                s2c = work.tile([128, 32], F32, tag="s2c")
                nc.vector.tensor_tensor(s2c, sfc, sig, ALU.add)
                c_new = state.tile([128, 32], F32, tag="c")
                nc.vector.tensor_scalar_mul(c_new, s2c, 0.5)
                tc_ = work.tile([128, 32], F32, tag="tc")
                nc.scalar.activation(tc_, s2c, AF.Tanh, scale=0.5)
                # h state kept as 2h (W_R/W_hh pre-halved on host); fp16 direct
                h_new16 = state.tile([128, 32], F16, tag="h16")
                nc.vector.scalar_tensor_tensor(
                    h_new16, ts_a[:, 64:96], 1.0, tc_, ALU.add, ALU.mult)
                nc.vector.tensor_scalar_mul(
                    hbuf[:, ts_].rearrange("p k b -> p (k b)"), h_new16, 0.5)

                if debug and t == dbg_t:
                    nc.sync.dma_start(out=dbg_rvt[:, :], in_=rvt)
                    nc.sync.dma_start(out=dbg_e[:, :, :], in_=e_all)
                    nc.sync.dma_start(out=dbg_expb[:, :], in_=expb)
                    nc.sync.dma_start(out=dbg_rhsbd[:, :], in_=rhs_bd)
                    nc.sync.dma_start(out=dbg_hts[:, :], in_=h_ts)
                    nc.sync.dma_start(out=dbg_gates[:, :], in_=gates)
                    nc.sync.dma_start(out=dbg_hprev[:, :], in_=h_prev)
                    nc.sync.dma_start(out=dbg_cprev[:, :], in_=c_prev)

                c_prev = c_new
                h_prev = h_new16
                if ts_ == HB - 1 or t == T - 1:
                    nb = ts_ + 1
                    t0 = t - nb + 1
                    nc.sync.dma_start(
                        out=out_c[:, t0:t0 + nb, :, :], in_=hbuf[:, :nb])
            psumg_pool.release()
            psum.release()
    nc.finalize()
    return nc


# ---------------- host side ----------------

# gate permutation: torch order (i, f, g, o) -> kernel order (i, f, o, g)
def permute_gates(w):
    """w: [4H, ...] rows in (i,f,g,o) order -> (i,f,o,g) order."""
    i, f, g, o = np.split(w, 4, axis=0)
    return np.concatenate([i, f, o, g], axis=0)


def prep_core_inputs(h_v, h_s, W, T=T_FULL):
    f16 = np.float16
    WS_T = np.ascontiguousarray(W["W_S"].T).astype(f16)
    WV_T = np.ascontiguousarray(W["W_V"].T).astype(f16)
    # gate scaling: g-rows doubled (single tanh(x/2) Act call recovers tanh(g));
    # W_R/W_hh halved because the h state is stored as 2h.
    gsc = np.ones((4 * H, 1), np.float32)
    gsc[3 * H:] = 2.0
    Wih_p = permute_gates(W["W_ih"]) * gsc
    Whh_p = permute_gates(W["W_hh"]) * gsc * 0.5
    bih_p = (permute_gates((W["b_ih"] + W["b_hh"])[:, None]) * gsc)[:, 0]
    WihV_T = np.ascontiguousarray(Wih_p[:, :DV].T).astype(f16)
    WihS_T = np.ascontiguousarray(Wih_p[:, DV:].T).astype(f16)
    Whh_T = np.ascontiguousarray(Whh_p.T).astype(f16)
    WR_T = np.ascontiguousarray(W["W_R"].T).astype(f16) * f16(0.5)
    wvec = np.ascontiguousarray(W["W_w"][0][:, None]).astype(f16)
    biasRSV = np.ascontiguousarray(
        (W["b_S"] + W["b_R"] + W["b_V"]).reshape(KH, 128).T).astype(np.float32)
    biasIH = np.ascontiguousarray(bih_p.reshape(KM, 128).T).astype(np.float32)
    bw = np.ascontiguousarray(W["b_w"].reshape(1, 1)).astype(np.float32)
    maps = []
    for c in range(NCORES):
        bs = slice(c * BLOC, (c + 1) * BLOC)
        hvT = np.ascontiguousarray(
            h_v[bs, :T].transpose(2, 1, 0).reshape(DV, T * BLOC)).astype(f16)
        hsT = np.ascontiguousarray(
            h_s[bs].transpose(2, 0, 1).reshape(DS, BLOC * L)).astype(f16)
        # per-b L-on-partitions layout: hs_bT[l, b, d] = h_s[b, l, d]
        hs_b = np.ascontiguousarray(
            h_s[bs].transpose(1, 0, 2)).astype(f16)  # [L, BLOC, DS]
        maps.append({
            "hvT": hvT, "hsT": hsT, "hs_bT": hs_b, "WS_T": WS_T, "WV_T": WV_T,
            "WihV_T": WihV_T, "WihS_T": WihS_T, "Whh_T": Whh_T, "WR_T": WR_T,
            "wvec": wvec, "biasRSV": biasRSV, "biasIH": biasIH, "bw": bw,
        })
    return maps


_NC_CACHE = {}


def kernel(**inputs):
    h_v = np.asarray(inputs["h_v"], dtype=np.float32)
    h_s = np.asarray(inputs["h_s"], dtype=np.float32)
    W = {k: np.asarray(v, dtype=np.float32) for k, v in inputs.items()}
    key = "full"
    if key not in _NC_CACHE:
        _NC_CACHE[key] = build_nc(T=T_FULL, Tc=16)
    nc = _NC_CACHE[key]
    maps = prep_core_inputs(h_v, h_s, W, T=T_FULL)
    res = run_bass_kernel_spmd(nc, maps, list(range(NCORES)))
    outs = []
    for c in range(NCORES):
        arr = res.results[c]["out_c"]  # [128, T, KH, BLOC]
        outs.append(np.ascontiguousarray(
            arr.transpose(3, 1, 2, 0)).reshape(BLOC, T_FULL, H))
    return np.concatenate(outs, axis=0).astype(np.float32)


if __name__ == "__main__":
    nc = build_nc(T=8, Tc=4)
    print("built ok")
